# revision 8
# baseline (speedup 1.0000x reference)
"""Trainium2 Bass kernel: 16-head MHA (B=4, S=2048, E=1024, Dh=64), 8 cores.

Sharding: core c handles batch b = c//2 and head-group g = c%2 (8 heads).
Each core computes its 8 heads' attention plus the partial output
projection in transposed layout oT[e, s]; the host sums the two
head-group partials per batch, transposes, and adds bo.

Per-core dataflow (matmuls bf16, fp32 PSUM accumulation):
  qT/kT[d, s]  = Wq/Wk.T @ xT          (per head-pair, d stacked 2x64)
  v[t, hd]     = xT.T @ Wv + ones.T@bv (natural layout, + ones col for colsum)
  scoresT[t,s] = kT.T @ qT   (2 heads row-tiled at PE rows 0-63/64-127,
                              concurrent)
  expT         = exp(0.125 * scoresT)  (ScalarE, cast to bf16)
  zT_un[d,s],colsum[s] = v_aug.T @ expT  (M=65: row 64 = colsum)
  zT           = zT_un * bcast(1/colsum) (bcast via K=1 matmul; both heads'
                 broadcasts col-tiled into one PSUM tile, one approx recip)
  oT[e, s]    += Wo_h.T @ zT_h  (accumulated over the core's 8 heads)

Scheduling: the Activation engine (256 exp tiles x ~1.1us) is the
bottleneck; emission is a flat slot loop (one slot per (pair, t-chunk))
that keeps ACT saturated: scores+exp lead each slot, the attention-V
matmuls trail ~8 slots behind, and all projection / out-projection /
normalization work is spread into the remaining PE slack via an
earliest-deadline filler queue, so the PE never idles long enough to
re-throttle (HAM) and ACT never starves.
"""

import numpy as np
import ml_dtypes

B, S, E = 4, 2048, 1024
H, Dh = 16, 64
N_CORES = 8
HPC = 8          # heads per core
MP = 4           # head-pairs per core
SC, SCW = 4, 512  # s-chunks
TC, TCW = 16, 128  # t-chunks
KE = 8           # k-tiles over E
ECN = 8          # e-chunks of 128 (outT partition tiles)
NPAIR = SC * MP   # 16 (sc, m) pairs, sc-major
NSLOT = NPAIR * TC

BF16 = ml_dtypes.bfloat16

_PROG = None


def _build_program(repeats=None, timing=False, parts=3):
    """Emit the Bass/Tile program. Returns (nc, names_dict).

    repeats: if set, wrap the whole body in a For_i loop (for marginal
    per-iteration HW timing; not used by the graded kernel() path).
    timing: demote the real output to internal DRAM and expose a tiny
    dummy output instead, so timing calls don't pay output transfers.
    """
    from contextlib import ExitStack

    import concourse.mybir as mybir
    import concourse.tile as tile
    from concourse import bacc

    dt = mybir.dt
    AF = mybir.ActivationFunctionType
    OP = mybir.AluOpType

    nc = bacc.Bacc(None, target_bir_lowering=False, debug=False)
    with tile.TileContext(nc) as tc:
        with tc.tile_pool(name="dram", bufs=1, space="DRAM") as dram:
            xT_d = dram.tile([E, S], dt.bfloat16, kind="ExternalInput")
            wq_d = dram.tile([E, HPC * Dh], dt.bfloat16, kind="ExternalInput")
            wk_d = dram.tile([E, HPC * Dh], dt.bfloat16, kind="ExternalInput")
            wv_d = dram.tile([E, HPC * Dh], dt.bfloat16, kind="ExternalInput")
            wo_d = dram.tile([128, MP, E], dt.bfloat16, kind="ExternalInput")
            bq_d = dram.tile([128, MP], dt.float32, kind="ExternalInput")
            bk_d = dram.tile([128, MP], dt.float32, kind="ExternalInput")
            bv_d = dram.tile([1, HPC * Dh], dt.bfloat16, kind="ExternalInput")
            if timing:
                oT_d = dram.tile([E, S], dt.float32, kind="Internal")
                dummy_d = dram.tile([1, 4], dt.bfloat16, kind="ExternalOutput")
            else:
                oT_d = dram.tile([E, S], dt.float32, kind="ExternalOutput")
                dummy_d = None

            with (
                tc.tile_pool(name="const", bufs=1) as const,
                tc.tile_pool(name="expp", bufs=32) as expp,
                tc.tile_pool(name="zpool", bufs=2) as zpool,
                tc.tile_pool(name="work", bufs=2) as work,
                tc.tile_pool(name="norm1", bufs=2) as norm1,
                tc.tile_pool(name="psum_sT", bufs=2, space="PSUM") as psum_sT,
                tc.tile_pool(name="psum_av", bufs=2, space="PSUM") as psum_av,
                tc.tile_pool(name="psum_rot", bufs=2, space="PSUM") as psum_rot,
                ExitStack() as _es,
            ):
                if repeats is not None:
                    _es.enter_context(tc.For_i(
                        0, repeats, 1,
                        hint_engines=(
                            mybir.EngineType.PE, mybir.EngineType.Activation,
                            mybir.EngineType.DVE, mybir.EngineType.SP,
                            mybir.EngineType.Pool,
                        ),
                    ))
                # ---- persistent SBUF ----
                xT = const.tile([128, KE, S], dt.bfloat16)
                wq = const.tile([128, KE, HPC * Dh], dt.bfloat16)
                wk = const.tile([128, KE, HPC * Dh], dt.bfloat16)
                wv = const.tile([128, KE, HPC * Dh], dt.bfloat16)
                wo = const.tile([128, MP, E], dt.bfloat16)
                bqk = const.tile([128, 2 * MP], dt.float32)
                onesbv = const.tile([1, HPC * Dh + 128], dt.bfloat16)
                qT2 = const.tile([128, MP, S], dt.bfloat16)
                kT2 = const.tile([128, MP, S], dt.bfloat16)
                v_sb = const.tile([128, TC, HPC, Dh + 1], dt.bfloat16)

                # ---- input DMAs (xT first: it gates the first scores) ----
                xTr = xT_d[:].rearrange("(a p) c -> p a c", p=128)
                for k in range(KE):
                    nc.sync.dma_start(xT[:, k:k + 1, :], xTr[:, k:k + 1, :])
                nc.sync.dma_start(wq[:, :, :], wq_d[:].rearrange("(a p) c -> p a c", p=128))
                nc.sync.dma_start(wk[:, :, :], wk_d[:].rearrange("(a p) c -> p a c", p=128))
                nc.sync.dma_start(bqk[:, 0:MP], bq_d[:])
                nc.sync.dma_start(bqk[:, MP:2 * MP], bk_d[:])
                nc.sync.dma_start(wv[:, :, :], wv_d[:].rearrange("(a p) c -> p a c", p=128))
                nc.sync.dma_start(onesbv[0:1, 0:HPC * Dh], bv_d[:])
                nc.vector.memset(onesbv[0:1, HPC * Dh:], 1.0)
                nc.vector.memset(v_sb[:, :, :, Dh:Dh + 1], 1.0)
                nc.sync.dma_start(wo[:, :, :], wo_d[:])
                if dummy_d is not None:
                    nc.sync.dma_start(dummy_d[:, :], onesbv[0:1, 0:4])

                # =========================================================
                # Work-unit generators (each unit emits ~1 matmul; groups
                # share a rotating PSUM tile held across their units).
                # =========================================================
                MM_CYC = 530      # ~N=512 matmul issue cost in PE cycles

                def proj_units(w_sb, boff, dst, m, sc):
                    """8 accumulating MMs + bias-add evict for q/k chunk."""
                    ssl = slice(sc * SCW, (sc + 1) * SCW)
                    state = {}

                    def mk(k):
                        def u():
                            if k == 0:
                                state["p"] = psum_rot.tile(
                                    [128, SCW], dt.float32, tag="rot",
                                    name=f"pr_{boff}_{m}_{sc}")
                            nc.tensor.matmul(
                                state["p"][:, :],
                                w_sb[:, k, m * 128:(m + 1) * 128],
                                xT[:, k, ssl],
                                start=(k == 0), stop=(k == KE - 1),
                            )
                            if k == KE - 1:
                                nc.vector.tensor_scalar_add(
                                    dst[:, m, ssl], state["p"][:, :],
                                    bqk[:, boff + m:boff + m + 1])
                        return u
                    return [mk(k) for k in range(KE)]

                def vproj_units(t):
                    """8 accumulating MMs + bias MM + evict for v t-chunk."""
                    tsl = slice(t * TCW, (t + 1) * TCW)
                    state = {}

                    def mk(k):
                        def u():
                            if k == 0:
                                state["p"] = psum_rot.tile(
                                    [128, HPC * Dh], dt.float32, tag="rot",
                                    name=f"pv_{t}")
                            nc.tensor.matmul(
                                state["p"][:, :], xT[:, k, tsl], wv[:, k, :],
                                start=(k == 0), stop=False,
                            )
                            if k == KE - 1:
                                nc.tensor.matmul(
                                    state["p"][:, :],
                                    onesbv[0:1, HPC * Dh:HPC * Dh + 128],
                                    onesbv[0:1, 0:HPC * Dh],
                                    start=False, stop=True,
                                )
                                nc.vector.tensor_copy(
                                    v_sb[:, t, :, 0:Dh],
                                    state["p"][:, :].rearrange(
                                        "p (h c) -> p h c", c=Dh),
                                )
                        return u
                    return [mk(k) for k in range(KE)]

                def outproj_units(sc, ec, zT2):
                    """4 accumulating MMs + evict copy + output DMA."""
                    ssl = slice(sc * SCW, (sc + 1) * SCW)
                    state = {}

                    def mk(m):
                        def u():
                            if m == 0:
                                state["p"] = psum_rot.tile(
                                    [128, SCW], dt.float32, tag="rot",
                                    name=f"po_{sc}_{ec}")
                            nc.tensor.matmul(
                                state["p"][:, :],
                                wo[:, m, ec * 128:(ec + 1) * 128],
                                zT2[:, m, :],
                                start=(m == 0), stop=(m == MP - 1),
                            )
                            if m == MP - 1:
                                ob = work.tile([128, SCW], dt.float32,
                                               tag="ob", name=f"ob_{sc}_{ec}")
                                nc.vector.tensor_copy(ob[:, :], state["p"][:, :])
                                nc.sync.dma_start(
                                    oT_d[ec * 128:(ec + 1) * 128, ssl],
                                    ob[:, :])
                        return u
                    return [mk(m) for m in range(MP)]

                # =========================================================
                # EDF filler queue
                # =========================================================
                filler = []   # list of [due_slot, seq_no, units_list, idx]
                seq_counter = [0]

                def add_group(due, units):
                    g = [due, seq_counter[0], units, 0]
                    filler.append(g)
                    seq_counter[0] += 1
                    filler.sort(key=lambda g_: (g_[0], g_[1]))
                    return g

                def pop_filler_unit():
                    """Emit one unit from the earliest-deadline group.
                    Stays on the same group until it completes (bounds the
                    number of concurrently-open PSUM groups)."""
                    while filler and filler[0][3] >= len(filler[0][2]):
                        filler.pop(0)
                    if not filler:
                        return False
                    g = filler[0]
                    g[2][g[3]]()   # may mutate/re-sort filler (add_group)
                    g[3] += 1
                    if g[3] == len(g[2]):
                        try:
                            filler.remove(g)
                        except ValueError:
                            pass
                    return True

                def force_group(g):
                    while g[3] < len(g[2]):
                        g[2][g[3]]()
                        g[3] += 1

                def force_due(slot):
                    while filler:
                        while filler and filler[0][3] >= len(filler[0][2]):
                            filler.pop(0)
                        if not filler or filler[0][0] > slot:
                            break
                        pop_filler_unit()

                # q/k/v groups with deadlines (slot of first use, less a
                # small margin so they normally drain early via slack).
                # sc-major pair order => pairs[i] has m = i % MP, so head
                # pair m's kT2 chunk c is first used at slot 16*m + 4*c.
                for m in range(MP):
                    for c in range(SC):
                        due = 16 * m + 4 * c - 2 if (m, c) != (0, 0) else -1
                        add_group(due, proj_units(wk, MP, kT2, m, c))
                for i, (sc, m) in enumerate(
                        [(s_, m_) for s_ in range(SC) for m_ in range(MP)]):
                    add_group(16 * i - 2 if i else -1,
                              proj_units(wq, 0, qT2, m, sc))
                vdone = [False] * TC
                vgroups = []
                for t in range(TC):
                    units = vproj_units(t)
                    units.append(lambda t=t: vdone.__setitem__(t, True))
                    vgroups.append(add_group(10 + 2 * t, units))

                # =========================================================
                # Per-slot structural pieces
                # =========================================================
                pairs = [(s_, m_) for s_ in range(SC) for m_ in range(MP)]
                etiles = [[None] * TC for _ in range(NPAIR)]
                pz_tiles = [None] * NPAIR
                zT2s = {}

                def emit_scores_exp(i, t):
                    sc, m = pairs[i]
                    ssl = slice(sc * SCW, (sc + 1) * SCW)
                    pst = psum_sT.tile([128, 2 * SCW], dt.float32, tag="sT")
                    for j in range(2):
                        hoff = j * Dh
                        nc.tensor.matmul(
                            pst[:, j * SCW:(j + 1) * SCW],
                            kT2[hoff:hoff + Dh, m, t * TCW:(t + 1) * TCW],
                            qT2[hoff:hoff + Dh, m, ssl],
                            start=True, stop=True,
                            tile_position=(hoff, 0),
                        )
                    e = expp.tile([128, 2, SCW], dt.bfloat16, tag="e")
                    nc.scalar.activation(e[:, :, :], pst[:, :], AF.Exp,
                                         scale=0.125)
                    etiles[i][t] = e

                def emit_av(i, t):
                    sc, m = pairs[i]
                    if t == 0:
                        pz_tiles[i] = (
                            psum_av.tile([Dh + 1, SCW], dt.float32, tag="pz",
                                         name=f"pze_{i}"),
                            psum_av.tile([Dh + 1, SCW], dt.float32, tag="pz",
                                         name=f"pzo_{i}"),
                        )
                    pz_e, pz_o = pz_tiles[i]
                    e = etiles[i][t]
                    nc.tensor.matmul(pz_e[:, :], v_sb[:, t, 2 * m, :],
                                     e[:, 0, :],
                                     start=(t == 0), stop=(t == TC - 1))
                    nc.tensor.matmul(pz_o[:, :], v_sb[:, t, 2 * m + 1, :],
                                     e[:, 1, :],
                                     start=(t == 0), stop=(t == TC - 1))
                    etiles[i][t] = None

                av_norm_done = [-1]  # highest pair whose norm TT is emitted

                def emit_norm_a(i, cur_slot):
                    """Phase A at AV(i) completion: colsum copies (DVE only),
                    then queue phase B (PE broadcast + recip + scale) as a
                    near-due filler group so the PE never sits on the DVE
                    latency of the colsum copies."""
                    sc, m = pairs[i]
                    if m == 0:
                        zT2s[sc] = zpool.tile([128, MP, SCW], dt.bfloat16,
                                              tag="zT", name=f"zT_{sc}")
                    zT2 = zT2s[sc]
                    pz_e, pz_o = pz_tiles[i]
                    cs_e = norm1.tile([1, SCW], dt.bfloat16, tag="cse",
                                      name=f"cse_{i}")
                    cs_o = norm1.tile([1, SCW], dt.bfloat16, tag="cso",
                                      name=f"cso_{i}")
                    nc.vector.tensor_copy(cs_e[0:1, :], pz_e[Dh:Dh + 1, :])
                    nc.vector.tensor_copy(cs_o[0:1, :], pz_o[Dh:Dh + 1, :])
                    state = {}

                    def u_bcast():
                        state["pbc"] = psum_rot.tile(
                            [128, SCW], dt.float32, tag="rot", name=f"pbc_{i}")
                        ones = onesbv[0:1, HPC * Dh:HPC * Dh + Dh]
                        nc.tensor.matmul(state["pbc"][0:Dh, :], ones,
                                         cs_e[0:1, :], start=True, stop=True)
                        nc.tensor.matmul(state["pbc"][Dh:2 * Dh, :], ones,
                                         cs_o[0:1, :], start=True, stop=True,
                                         tile_position=(0, Dh))

                    def u_scale():
                        bch = norm1.tile([128, SCW], dt.float32, tag="bch",
                                         name=f"bch_{i}")
                        nc.vector.reciprocal_approx_fast(
                            bch[:, :], state["pbc"][:, :])
                        nc.vector.tensor_tensor(
                            zT2[0:Dh, m, :], pz_e[0:Dh, :], bch[0:Dh, :],
                            OP.mult)
                        ztmp = work.tile([Dh, SCW], dt.bfloat16, tag="ztmp",
                                         name=f"zt_{i}")
                        nc.vector.tensor_tensor(
                            ztmp[:, :], pz_o[0:Dh, :], bch[Dh:2 * Dh, :],
                            OP.mult)
                        nc.sync.dma_start(zT2[Dh:2 * Dh, m, :], ztmp[:, :])
                        av_norm_done[0] = max(av_norm_done[0], i)
                        if m == MP - 1 and parts >= 3:
                            for ec in range(ECN):
                                add_group(16 * (i + 4) + 8 if i + 1 < NPAIR
                                          else NSLOT,
                                          outproj_units(sc, ec, zT2))

                    add_group(cur_slot + 1, [u_bcast, u_scale])

                # =========================================================
                # Slot loop
                # =========================================================
                SLOT_BUDGET = 2600   # PE cycles per ACT cadence (~1.1us)
                MAX_LAG = 24         # hard cap < expp bufs (deadlock guard)
                av_next = [0, 0]     # (pair, t) head of the AV queue

                def av_gate(SL):
                    """None if the next AV t-chunk may emit, else the blocker:
                    'v' (forceable) or 'done'/'exp'/'pz' (not)."""
                    ip, tp = av_next
                    if ip >= NPAIR:
                        return "done"
                    if 16 * ip + tp >= SL:       # its exp not yet emitted
                        return "exp"
                    if tp == 0 and ip > 0 and av_norm_done[0] < ip - 1:
                        return "pz"             # pz banks still held
                    if not vdone[tp]:
                        return "v"
                    return None

                def emit_av_step(SL, cap, force_v=False):
                    n = 0
                    while n < cap:
                        blocker = av_gate(SL)
                        if blocker == "v" and force_v:
                            force_group(vgroups[av_next[1]])
                            blocker = None
                        if blocker is not None:
                            break
                        ip, tp = av_next
                        emit_av(ip, tp)
                        n += 1
                        if tp == TC - 1:
                            av_next[0], av_next[1] = ip + 1, 0
                            emit_norm_a(ip, SL)
                        else:
                            av_next[1] += 1
                    return n

                for SL in range(NSLOT):
                    i, t = divmod(SL, TC)
                    used = 0
                    # forced: deadline work (this slot's k/q dependencies,
                    # pending norm phase-B groups)
                    force_due(SL)
                    # hard AV-lag cap: the exp-tile pool is finite and the
                    # ACT->PE dependency chain deadlocks past it
                    lag = SL - (16 * av_next[0] + av_next[1])
                    if lag >= MAX_LAG:
                        used += 1060 * emit_av_step(SL, lag - MAX_LAG + 4,
                                                    force_v=True)
                    emit_scores_exp(i, t)
                    used += 560
                    # AV pacing: trail ~8 slots; catch up harder at the end
                    target = SL - 8 if i < NPAIR - 1 else SL - 4
                    cap = 2 if i < NPAIR - 1 else 3
                    deficit = target - (16 * av_next[0] + av_next[1])
                    if deficit > 0:
                        used += 1060 * emit_av_step(SL, min(cap, deficit))
                    # discretionary filler up to the slot budget
                    while used < SLOT_BUDGET:
                        if not pop_filler_unit():
                            break
                        used += MM_CYC

                # =========================================================
                # Tail: drain AV, final norms, remaining out-projections
                # =========================================================
                guard = 0
                while av_next[0] < NPAIR or filler:
                    progressed = emit_av_step(NSLOT, 16, force_v=True) > 0
                    if pop_filler_unit():
                        progressed = True
                    if not progressed:
                        guard += 1
                        if guard > 4:
                            raise RuntimeError(
                                f"emitter wedged: av={av_next}, "
                                f"filler={len(filler)}")
                    else:
                        guard = 0

    nc.compile()
    names = {
        "xT": xT_d.name, "wq": wq_d.name, "wk": wk_d.name, "wv": wv_d.name,
        "wo": wo_d.name, "bq": bq_d.name, "bk": bk_d.name, "bv": bv_d.name,
        "oT": oT_d.name,
    }
    return nc, names


def get_program():
    global _PROG
    if _PROG is None:
        _PROG = _build_program()
    return _PROG


def make_in_maps(x, Wq, bq, Wk, bk, Wv, bv, Wo, names):
    """Host-side sharding: per-core input dict (bf16 casts + layout prep)."""
    in_maps = []
    for c in range(N_CORES):
        b, g = divmod(c, 2)
        hsl = slice(g * HPC, (g + 1) * HPC)
        xT_c = np.ascontiguousarray(x[b].T).astype(BF16)                 # [E, S]
        wq_c = np.ascontiguousarray(
            Wq[hsl].transpose(1, 0, 2).reshape(E, HPC * Dh)).astype(BF16)
        wk_c = np.ascontiguousarray(
            Wk[hsl].transpose(1, 0, 2).reshape(E, HPC * Dh)).astype(BF16)
        wv_c = np.ascontiguousarray(
            Wv[hsl].transpose(1, 0, 2).reshape(E, HPC * Dh)).astype(BF16)
        # Wo rows for this head group, packed [Dh, HPC, E] (head on free axis)
        wo_c = np.ascontiguousarray(
            Wo[g * HPC * Dh:(g + 1) * HPC * Dh].reshape(MP, 128, E)
            .transpose(1, 0, 2)).astype(BF16)
        bq_c = np.ascontiguousarray(bq[hsl].reshape(MP, 128).T).astype(np.float32)
        bk_c = np.ascontiguousarray(bk[hsl].reshape(MP, 128).T).astype(np.float32)
        bv_c = bv[hsl].reshape(1, HPC * Dh).astype(BF16)
        in_maps.append({
            names["xT"]: xT_c, names["wq"]: wq_c, names["wk"]: wk_c,
            names["wv"]: wv_c, names["wo"]: wo_c, names["bq"]: bq_c,
            names["bk"]: bk_c, names["bv"]: bv_c,
        })
    return in_maps


def combine_outputs(results, bo, names):
    """Host-side unshard: sum head-group partials, transpose, add bo."""
    out = np.empty((B, S, E), np.float32)
    for b in range(B):
        oT = results[2 * b][names["oT"]] + results[2 * b + 1][names["oT"]]
        out[b] = oT.T + bo
    return out


_RUNNER = None


def _make_runner(nc):
    """Cached jit callable running `nc` SPMD on 8 cores via PJRT/axon.
    Mirrors run_bass_via_pjrt but is built once and reused across calls."""
    import jax
    from jax.sharding import Mesh, PartitionSpec
    try:
        from jax.experimental.shard_map import shard_map
    except ImportError:
        from jax import shard_map
    import concourse.mybir as mybir
    from concourse import bass2jax

    bass2jax.install_neuronx_cc_hook()
    pid_name = nc.partition_id_tensor.name if nc.partition_id_tensor else None
    in_names, out_names, out_avals, out_shapes = [], [], [], []
    for alloc in nc.m.functions[0].allocations:
        if not isinstance(alloc, mybir.MemoryLocationSet):
            continue
        name = alloc.memorylocations[0].name
        if alloc.kind == "ExternalInput" and name != pid_name:
            in_names.append(name)
        elif alloc.kind == "ExternalOutput":
            shape = tuple(alloc.tensor_shape)
            dtype = mybir.dt.np(alloc.dtype)
            out_names.append(name)
            out_avals.append(jax.core.ShapedArray(shape, dtype))
            out_shapes.append((shape, dtype))
    n_params = len(in_names)
    all_names = list(in_names) + list(out_names) + ([pid_name] if pid_name else [])

    def _body(*args):
        operands = list(args)
        if pid_name is not None:
            operands.append(bass2jax.partition_id_tensor())
        return tuple(bass2jax._bass_exec_p.bind(
            *operands, out_avals=tuple(out_avals), in_names=tuple(all_names),
            out_names=tuple(out_names), lowering_input_output_aliases=(),
            sim_require_finite=True, sim_require_nnan=True, nc=nc))

    devices = jax.devices()[:N_CORES]
    mesh = Mesh(np.asarray(devices), ("core",))
    nio = n_params + len(out_names)
    sharded = jax.jit(
        shard_map(_body, mesh=mesh, in_specs=(PartitionSpec("core"),) * nio,
                  out_specs=(PartitionSpec("core"),) * len(out_names),
                  check_rep=False),
        donate_argnums=tuple(range(n_params, nio)), keep_unused=True)

    def run(in_maps):
        concat_in = [
            np.concatenate([np.asarray(m[nm]) for m in in_maps], axis=0)
            for nm in in_names]
        zeros = [np.zeros((N_CORES * s[0], *s[1:]), dty)
                 for s, dty in out_shapes]
        outs = sharded(*concat_in, *zeros)
        return [
            {name: np.asarray(outs[i]).reshape(N_CORES, *out_shapes[i][0])[c]
             for i, name in enumerate(out_names)}
            for c in range(N_CORES)]

    return run


def kernel(x, Wq, bq, Wk, bk, Wv, bv, Wo, bo):
    global _RUNNER
    nc, names = get_program()
    in_maps = make_in_maps(
        np.asarray(x), np.asarray(Wq), np.asarray(bq), np.asarray(Wk),
        np.asarray(bk), np.asarray(Wv), np.asarray(bv), np.asarray(Wo), names,
    )
    try:
        if _RUNNER is None:
            _RUNNER = _make_runner(nc)
        results = _RUNNER(in_maps)
    except Exception:
        from concourse.bass_utils import run_bass_kernel_spmd
        _RUNNER = None
        results = run_bass_kernel_spmd(
            nc, in_maps, core_ids=list(range(N_CORES))).results
    return combine_outputs(results, np.asarray(bo, np.float32), names)


# revision 9
# speedup vs baseline: 1.4282x; 1.4282x over previous
"""Trainium2 Bass kernel: 16-head MHA (B=4, S=2048, E=1024, Dh=64), 8 cores.

Sharding: core c handles batch b = c//2 and head-group g = c%2 (8 heads).
Each core computes its 8 heads' attention plus the partial output
projection in transposed layout oT[e, s]; the host sums the two
head-group partials per batch, transposes, and adds bo.

Per-core dataflow (matmuls bf16, fp32 PSUM accumulation):
  qT/kT[d, s]  = Wq/Wk.T @ xT          (per head-pair, d stacked 2x64)
  v[t, hd]     = xT.T @ Wv + ones.T@bv (natural layout, + ones col for colsum)
  scoresT[t,s] = kT.T @ qT   (2 heads row-tiled at PE rows 0-63/64-127,
                              concurrent)
  expT         = exp(0.125 * scoresT)  (ScalarE, cast to bf16)
  zT_un[d,s],colsum[s] = v_aug.T @ expT  (M=65: row 64 = colsum)
  zT           = zT_un * bcast(1/colsum) (bcast via K=1 matmul; both heads'
                 broadcasts col-tiled into one PSUM tile, one approx recip)
  oT[e, s]    += Wo_h.T @ zT_h  (accumulated over the core's 8 heads)

Scheduling: the Activation engine (256 exp tiles x ~1.1us) is the
bottleneck; emission is a flat slot loop (one slot per (pair, t-chunk))
that keeps ACT saturated: scores+exp lead each slot, the attention-V
matmuls trail ~8 slots behind, and all projection / out-projection /
normalization work is spread into the remaining PE slack via an
earliest-deadline filler queue, so the PE never idles long enough to
re-throttle (HAM) and ACT never starves.
"""

import numpy as np
import ml_dtypes

B, S, E = 4, 2048, 1024
H, Dh = 16, 64
N_CORES = 8
HPC = 8          # heads per core
MP = 4           # head-pairs per core
SC, SCW = 4, 512  # s-chunks
TC, TCW = 16, 128  # t-chunks
KE = 8           # k-tiles over E
ECN = 8          # e-chunks of 128 (outT partition tiles)
NPAIR = SC * MP   # 16 (sc, m) pairs, sc-major
NSLOT = NPAIR * TC

BF16 = ml_dtypes.bfloat16

_PROG = None


def _build_program(repeats=None, timing=False, parts=3):
    """Emit the Bass/Tile program. Returns (nc, names_dict).

    repeats: if set, wrap the whole body in a For_i loop (for marginal
    per-iteration HW timing; not used by the graded kernel() path).
    timing: demote the real output to internal DRAM and expose a tiny
    dummy output instead, so timing calls don't pay output transfers.
    """
    from contextlib import ExitStack

    import concourse.mybir as mybir
    import concourse.tile as tile
    from concourse import bacc

    dt = mybir.dt
    AF = mybir.ActivationFunctionType
    OP = mybir.AluOpType

    nc = bacc.Bacc(None, target_bir_lowering=False, debug=False)
    with tile.TileContext(nc) as tc:
        with tc.tile_pool(name="dram", bufs=1, space="DRAM") as dram:
            xT_d = dram.tile([E, S], dt.bfloat16, kind="ExternalInput")
            wq_d = dram.tile([E, HPC * Dh], dt.bfloat16, kind="ExternalInput")
            wk_d = dram.tile([E, HPC * Dh], dt.bfloat16, kind="ExternalInput")
            wv_d = dram.tile([E, HPC * Dh], dt.bfloat16, kind="ExternalInput")
            wo_d = dram.tile([128, MP, E], dt.bfloat16, kind="ExternalInput")
            bq_d = dram.tile([128, MP], dt.float32, kind="ExternalInput")
            bk_d = dram.tile([128, MP], dt.float32, kind="ExternalInput")
            bv_d = dram.tile([1, HPC * Dh], dt.bfloat16, kind="ExternalInput")
            if timing:
                oT_d = dram.tile([E, S], dt.float32, kind="Internal")
                dummy_d = dram.tile([1, 4], dt.bfloat16, kind="ExternalOutput")
            else:
                oT_d = dram.tile([E, S], dt.float32, kind="ExternalOutput")
                dummy_d = None

            with (
                tc.tile_pool(name="const", bufs=1) as const,
                tc.tile_pool(name="expp", bufs=32) as expp,
                tc.tile_pool(name="zpool", bufs=2) as zpool,
                tc.tile_pool(name="work", bufs=2) as work,
                tc.tile_pool(name="norm1", bufs=2) as norm1,
                tc.tile_pool(name="psum_sT", bufs=2, space="PSUM") as psum_sT,
                tc.tile_pool(name="psum_av", bufs=2, space="PSUM") as psum_av,
                tc.tile_pool(name="psum_rot", bufs=2, space="PSUM") as psum_rot,
                ExitStack() as _es,
            ):
                # ---- loop-invariant SBUF + weight DMAs (outside For_i) ----
                wq = const.tile([128, KE, HPC * Dh], dt.bfloat16)
                wk = const.tile([128, KE, HPC * Dh], dt.bfloat16)
                wv = const.tile([128, KE, HPC * Dh], dt.bfloat16)
                wo = const.tile([128, MP, E], dt.bfloat16)
                bqk = const.tile([128, 2 * MP], dt.float32)
                onesbv = const.tile([1, HPC * Dh + 128], dt.bfloat16)
                v_sb = const.tile([128, TC, HPC, Dh + 1], dt.bfloat16)
                nc.sync.dma_start(wq[:, :, :], wq_d[:].rearrange("(a p) c -> p a c", p=128))
                nc.sync.dma_start(wk[:, :, :], wk_d[:].rearrange("(a p) c -> p a c", p=128))
                nc.sync.dma_start(bqk[:, 0:MP], bq_d[:])
                nc.sync.dma_start(bqk[:, MP:2 * MP], bk_d[:])
                nc.sync.dma_start(wv[:, :, :], wv_d[:].rearrange("(a p) c -> p a c", p=128))
                nc.sync.dma_start(onesbv[0:1, 0:HPC * Dh], bv_d[:])
                nc.vector.memset(onesbv[0:1, HPC * Dh:], 1.0)
                nc.vector.memset(v_sb[:, :, :, Dh:Dh + 1], 1.0)
                nc.sync.dma_start(wo[:, :, :], wo_d[:])
                if repeats is not None:
                    _es.enter_context(tc.For_i(
                        0, repeats, 1,
                        hint_engines=(
                            mybir.EngineType.PE, mybir.EngineType.Activation,
                            mybir.EngineType.DVE, mybir.EngineType.SP,
                            mybir.EngineType.Pool,
                        ),
                    ))
                # ---- per-iteration SBUF + input DMA ----
                xT = const.tile([128, KE, S], dt.bfloat16)
                qT2 = const.tile([128, MP, S], dt.bfloat16)
                kT2 = const.tile([128, MP, S], dt.bfloat16)
                xTr = xT_d[:].rearrange("(a p) c -> p a c", p=128)
                for k in range(KE):
                    nc.sync.dma_start(xT[:, k:k + 1, :], xTr[:, k:k + 1, :])
                if dummy_d is not None:
                    nc.sync.dma_start(dummy_d[:, :], onesbv[0:1, 0:4])

                # =========================================================
                # Work-unit generators (each unit emits ~1 matmul; groups
                # share a rotating PSUM tile held across their units).
                # =========================================================
                MM_CYC = 530      # ~N=512 matmul issue cost in PE cycles

                def proj_units(w_sb, boff, dst, m, sc):
                    """8 accumulating MMs + bias-add evict for q/k chunk."""
                    ssl = slice(sc * SCW, (sc + 1) * SCW)
                    state = {}

                    def mk(k):
                        def u():
                            if k == 0:
                                state["p"] = psum_rot.tile(
                                    [128, SCW], dt.float32, tag="rot",
                                    name=f"pr_{boff}_{m}_{sc}")
                            nc.tensor.matmul(
                                state["p"][:, :],
                                w_sb[:, k, m * 128:(m + 1) * 128],
                                xT[:, k, ssl],
                                start=(k == 0), stop=(k == KE - 1),
                            )
                            if k == KE - 1:
                                nc.vector.tensor_scalar_add(
                                    dst[:, m, ssl], state["p"][:, :],
                                    bqk[:, boff + m:boff + m + 1])
                        return u
                    return [mk(k) for k in range(KE)]

                def vproj_units(t):
                    """8 accumulating MMs + bias MM + evict for v t-chunk."""
                    tsl = slice(t * TCW, (t + 1) * TCW)
                    state = {}

                    def mk(k):
                        def u():
                            if k == 0:
                                state["p"] = psum_rot.tile(
                                    [128, HPC * Dh], dt.float32, tag="rot",
                                    name=f"pv_{t}")
                            nc.tensor.matmul(
                                state["p"][:, :], xT[:, k, tsl], wv[:, k, :],
                                start=(k == 0), stop=False,
                            )
                            if k == KE - 1:
                                nc.tensor.matmul(
                                    state["p"][:, :],
                                    onesbv[0:1, HPC * Dh:HPC * Dh + 128],
                                    onesbv[0:1, 0:HPC * Dh],
                                    start=False, stop=True,
                                )
                                nc.vector.tensor_copy(
                                    v_sb[:, t, :, 0:Dh],
                                    state["p"][:, :].rearrange(
                                        "p (h c) -> p h c", c=Dh),
                                )
                        return u
                    return [mk(k) for k in range(KE)]

                def outproj_units(sc, ec, zT2):
                    """4 accumulating MMs + evict copy + output DMA."""
                    ssl = slice(sc * SCW, (sc + 1) * SCW)
                    state = {}

                    def mk(m):
                        def u():
                            if m == 0:
                                state["p"] = psum_rot.tile(
                                    [128, SCW], dt.float32, tag="rot",
                                    name=f"po_{sc}_{ec}")
                            nc.tensor.matmul(
                                state["p"][:, :],
                                wo[:, m, ec * 128:(ec + 1) * 128],
                                zT2[:, m, :],
                                start=(m == 0), stop=(m == MP - 1),
                            )
                            if m == MP - 1:
                                ob = work.tile([128, SCW], dt.float32,
                                               tag="ob", name=f"ob_{sc}_{ec}")
                                nc.vector.tensor_copy(ob[:, :], state["p"][:, :])
                                nc.sync.dma_start(
                                    oT_d[ec * 128:(ec + 1) * 128, ssl],
                                    ob[:, :])
                        return u
                    return [mk(m) for m in range(MP)]

                # =========================================================
                # EDF filler queue
                # =========================================================
                filler = []   # list of [due_slot, seq_no, units_list, idx]
                seq_counter = [0]

                def add_group(due, units):
                    g = [due, seq_counter[0], units, 0]
                    filler.append(g)
                    seq_counter[0] += 1
                    filler.sort(key=lambda g_: (g_[0], g_[1]))
                    return g

                def pop_filler_unit():
                    """Emit one unit from the earliest-deadline group.
                    Stays on the same group until it completes (bounds the
                    number of concurrently-open PSUM groups)."""
                    while filler and filler[0][3] >= len(filler[0][2]):
                        filler.pop(0)
                    if not filler:
                        return False
                    g = filler[0]
                    g[2][g[3]]()   # may mutate/re-sort filler (add_group)
                    g[3] += 1
                    if g[3] == len(g[2]):
                        try:
                            filler.remove(g)
                        except ValueError:
                            pass
                    return True

                def force_group(g):
                    while g[3] < len(g[2]):
                        g[2][g[3]]()
                        g[3] += 1

                def force_due(slot):
                    while filler:
                        while filler and filler[0][3] >= len(filler[0][2]):
                            filler.pop(0)
                        if not filler or filler[0][0] > slot:
                            break
                        pop_filler_unit()

                # q/k/v groups with deadlines (slot of first use, less a
                # small margin so they normally drain early via slack).
                # sc-major pair order => pairs[i] has m = i % MP, so head
                # pair m's kT2 chunk c is first used at slot 16*m + 4*c.
                for m in range(MP):
                    for c in range(SC):
                        due = 16 * m + 4 * c - 2 if (m, c) != (0, 0) else -1
                        add_group(due, proj_units(wk, MP, kT2, m, c))
                for i, (sc, m) in enumerate(
                        [(s_, m_) for s_ in range(SC) for m_ in range(MP)]):
                    due = min(16 * i - 2, 90 + 2 * i) if i else -1
                    add_group(due, proj_units(wq, 0, qT2, m, sc))
                vdone = [False] * TC
                vgroups = []
                for t in range(TC):
                    units = vproj_units(t)
                    units.append(lambda t=t: vdone.__setitem__(t, True))
                    vgroups.append(add_group(10 + 2 * t, units))

                # =========================================================
                # Per-slot structural pieces
                # =========================================================
                pairs = [(s_, m_) for s_ in range(SC) for m_ in range(MP)]
                etiles = [[None] * TC for _ in range(NPAIR)]
                pz_tiles = [None] * NPAIR
                zT2s = {}

                def emit_scores_exp(i, t):
                    sc, m = pairs[i]
                    ssl = slice(sc * SCW, (sc + 1) * SCW)
                    pst = psum_sT.tile([128, 2 * SCW], dt.float32, tag="sT")
                    for j in range(2):
                        hoff = j * Dh
                        nc.tensor.matmul(
                            pst[:, j * SCW:(j + 1) * SCW],
                            kT2[hoff:hoff + Dh, m, t * TCW:(t + 1) * TCW],
                            qT2[hoff:hoff + Dh, m, ssl],
                            start=True, stop=True,
                            tile_position=(hoff, 0),
                        )
                    e = expp.tile([128, 2, SCW], dt.bfloat16, tag="e")
                    nc.scalar.activation(e[:, :, :], pst[:, :], AF.Exp,
                                         scale=0.125)
                    etiles[i][t] = e

                def emit_av(i, t):
                    sc, m = pairs[i]
                    if t == 0:
                        pz_tiles[i] = (
                            psum_av.tile([Dh + 1, SCW], dt.float32, tag="pz",
                                         name=f"pze_{i}"),
                            psum_av.tile([Dh + 1, SCW], dt.float32, tag="pz",
                                         name=f"pzo_{i}"),
                        )
                    pz_e, pz_o = pz_tiles[i]
                    e = etiles[i][t]
                    nc.tensor.matmul(pz_e[:, :], v_sb[:, t, 2 * m, :],
                                     e[:, 0, :],
                                     start=(t == 0), stop=(t == TC - 1))
                    nc.tensor.matmul(pz_o[:, :], v_sb[:, t, 2 * m + 1, :],
                                     e[:, 1, :],
                                     start=(t == 0), stop=(t == TC - 1))
                    etiles[i][t] = None

                av_norm_done = [-1]  # highest pair whose norm TT is emitted

                def emit_norm_a(i, cur_slot):
                    """Phase A at AV(i) completion: colsum copies (DVE only),
                    then queue phase B (PE broadcast + recip + scale) as a
                    near-due filler group so the PE never sits on the DVE
                    latency of the colsum copies."""
                    sc, m = pairs[i]
                    if m == 0:
                        zT2s[sc] = zpool.tile([128, MP, SCW], dt.bfloat16,
                                              tag="zT", name=f"zT_{sc}")
                    zT2 = zT2s[sc]
                    pz_e, pz_o = pz_tiles[i]
                    cs_e = norm1.tile([1, SCW], dt.bfloat16, tag="cse",
                                      name=f"cse_{i}")
                    cs_o = norm1.tile([1, SCW], dt.bfloat16, tag="cso",
                                      name=f"cso_{i}")
                    nc.vector.tensor_copy(cs_e[0:1, :], pz_e[Dh:Dh + 1, :])
                    nc.vector.tensor_copy(cs_o[0:1, :], pz_o[Dh:Dh + 1, :])
                    state = {}

                    def u_bcast():
                        state["pbc"] = psum_rot.tile(
                            [128, SCW], dt.float32, tag="rot", name=f"pbc_{i}")
                        ones = onesbv[0:1, HPC * Dh:HPC * Dh + Dh]
                        nc.tensor.matmul(state["pbc"][0:Dh, :], ones,
                                         cs_e[0:1, :], start=True, stop=True)
                        nc.tensor.matmul(state["pbc"][Dh:2 * Dh, :], ones,
                                         cs_o[0:1, :], start=True, stop=True,
                                         tile_position=(0, Dh))

                    def u_scale():
                        bch = norm1.tile([128, SCW], dt.float32, tag="bch",
                                         name=f"bch_{i}")
                        nc.vector.reciprocal_approx_fast(
                            bch[:, :], state["pbc"][:, :])
                        nc.vector.tensor_tensor(
                            zT2[0:Dh, m, :], pz_e[0:Dh, :], bch[0:Dh, :],
                            OP.mult)
                        ztmp = work.tile([Dh, SCW], dt.bfloat16, tag="ztmp",
                                         name=f"zt_{i}")
                        nc.vector.tensor_tensor(
                            ztmp[:, :], pz_o[0:Dh, :], bch[Dh:2 * Dh, :],
                            OP.mult)
                        nc.sync.dma_start(zT2[Dh:2 * Dh, m, :], ztmp[:, :])
                        av_norm_done[0] = max(av_norm_done[0], i)
                        if m == MP - 1 and parts >= 3:
                            for ec in range(ECN):
                                add_group(16 * (i + 4) + 8 if i + 1 < NPAIR
                                          else NSLOT,
                                          outproj_units(sc, ec, zT2))

                    add_group(cur_slot + 1, [u_bcast, u_scale])

                # =========================================================
                # Slot loop
                # =========================================================
                SLOT_BUDGET = 2600   # PE cycles per ACT cadence (~1.1us)
                MAX_LAG = 24         # hard cap < expp bufs (deadlock guard)
                av_next = [0, 0]     # (pair, t) head of the AV queue

                def av_gate(SL):
                    """None if the next AV t-chunk may emit, else the blocker:
                    'v' (forceable) or 'done'/'exp'/'pz' (not)."""
                    ip, tp = av_next
                    if ip >= NPAIR:
                        return "done"
                    if 16 * ip + tp >= SL:       # its exp not yet emitted
                        return "exp"
                    if tp == 0 and ip > 0 and av_norm_done[0] < ip - 1:
                        return "pz"             # pz banks still held
                    if not vdone[tp]:
                        return "v"
                    return None

                def emit_av_step(SL, cap, force_v=False):
                    n = 0
                    while n < cap:
                        blocker = av_gate(SL)
                        if blocker == "v" and force_v:
                            force_group(vgroups[av_next[1]])
                            blocker = None
                        if blocker is not None:
                            break
                        ip, tp = av_next
                        emit_av(ip, tp)
                        n += 1
                        if tp == TC - 1:
                            av_next[0], av_next[1] = ip + 1, 0
                            emit_norm_a(ip, SL)
                        else:
                            av_next[1] += 1
                    return n

                for SL in range(NSLOT):
                    i, t = divmod(SL, TC)
                    used = 0
                    # forced: deadline work (this slot's k/q dependencies,
                    # pending norm phase-B groups)
                    force_due(SL)
                    # hard AV-lag cap: the exp-tile pool is finite and the
                    # ACT->PE dependency chain deadlocks past it
                    lag = SL - (16 * av_next[0] + av_next[1])
                    if lag >= MAX_LAG:
                        used += 1060 * emit_av_step(SL, lag - MAX_LAG + 4,
                                                    force_v=True)
                    emit_scores_exp(i, t)
                    used += 560
                    # AV pacing: trail ~8 slots; catch up harder at the end
                    target = SL - 8 if i < NPAIR - 1 else SL - 4
                    cap = 2 if i < NPAIR - 1 else 3
                    deficit = target - (16 * av_next[0] + av_next[1])
                    if deficit > 0:
                        used += 1060 * emit_av_step(SL, min(cap, deficit))
                    # discretionary filler up to the slot budget
                    while used < SLOT_BUDGET:
                        if not pop_filler_unit():
                            break
                        used += MM_CYC

                # =========================================================
                # Tail: drain AV, final norms, remaining out-projections
                # =========================================================
                guard = 0
                while av_next[0] < NPAIR or filler:
                    progressed = emit_av_step(NSLOT, 16, force_v=True) > 0
                    if pop_filler_unit():
                        progressed = True
                    if not progressed:
                        guard += 1
                        if guard > 4:
                            raise RuntimeError(
                                f"emitter wedged: av={av_next}, "
                                f"filler={len(filler)}")
                    else:
                        guard = 0

    nc.compile()
    names = {
        "xT": xT_d.name, "wq": wq_d.name, "wk": wk_d.name, "wv": wv_d.name,
        "wo": wo_d.name, "bq": bq_d.name, "bk": bk_d.name, "bv": bv_d.name,
        "oT": oT_d.name,
    }
    return nc, names


def get_program():
    global _PROG
    if _PROG is None:
        _PROG = _build_program()
    return _PROG


def make_in_maps(x, Wq, bq, Wk, bk, Wv, bv, Wo, names):
    """Host-side sharding: per-core input dict (bf16 casts + layout prep)."""
    in_maps = []
    for c in range(N_CORES):
        b, g = divmod(c, 2)
        hsl = slice(g * HPC, (g + 1) * HPC)
        xT_c = np.ascontiguousarray(x[b].T).astype(BF16)                 # [E, S]
        wq_c = np.ascontiguousarray(
            Wq[hsl].transpose(1, 0, 2).reshape(E, HPC * Dh)).astype(BF16)
        wk_c = np.ascontiguousarray(
            Wk[hsl].transpose(1, 0, 2).reshape(E, HPC * Dh)).astype(BF16)
        wv_c = np.ascontiguousarray(
            Wv[hsl].transpose(1, 0, 2).reshape(E, HPC * Dh)).astype(BF16)
        # Wo rows for this head group, packed [Dh, HPC, E] (head on free axis)
        wo_c = np.ascontiguousarray(
            Wo[g * HPC * Dh:(g + 1) * HPC * Dh].reshape(MP, 128, E)
            .transpose(1, 0, 2)).astype(BF16)
        bq_c = np.ascontiguousarray(bq[hsl].reshape(MP, 128).T).astype(np.float32)
        bk_c = np.ascontiguousarray(bk[hsl].reshape(MP, 128).T).astype(np.float32)
        bv_c = bv[hsl].reshape(1, HPC * Dh).astype(BF16)
        in_maps.append({
            names["xT"]: xT_c, names["wq"]: wq_c, names["wk"]: wk_c,
            names["wv"]: wv_c, names["wo"]: wo_c, names["bq"]: bq_c,
            names["bk"]: bk_c, names["bv"]: bv_c,
        })
    return in_maps


def combine_outputs(results, bo, names):
    """Host-side unshard: sum head-group partials, transpose, add bo."""
    out = np.empty((B, S, E), np.float32)
    for b in range(B):
        oT = results[2 * b][names["oT"]] + results[2 * b + 1][names["oT"]]
        out[b] = oT.T + bo
    return out


_RUNNER = None


def _make_runner(nc):
    """Cached jit callable running `nc` SPMD on 8 cores via PJRT/axon.
    Mirrors run_bass_via_pjrt but is built once and reused across calls."""
    import jax
    from jax.sharding import Mesh, PartitionSpec
    try:
        from jax.experimental.shard_map import shard_map
    except ImportError:
        from jax import shard_map
    import concourse.mybir as mybir
    from concourse import bass2jax

    bass2jax.install_neuronx_cc_hook()
    pid_name = nc.partition_id_tensor.name if nc.partition_id_tensor else None
    in_names, out_names, out_avals, out_shapes = [], [], [], []
    for alloc in nc.m.functions[0].allocations:
        if not isinstance(alloc, mybir.MemoryLocationSet):
            continue
        name = alloc.memorylocations[0].name
        if alloc.kind == "ExternalInput" and name != pid_name:
            in_names.append(name)
        elif alloc.kind == "ExternalOutput":
            shape = tuple(alloc.tensor_shape)
            dtype = mybir.dt.np(alloc.dtype)
            out_names.append(name)
            out_avals.append(jax.core.ShapedArray(shape, dtype))
            out_shapes.append((shape, dtype))
    n_params = len(in_names)
    all_names = list(in_names) + list(out_names) + ([pid_name] if pid_name else [])

    def _body(*args):
        operands = list(args)
        if pid_name is not None:
            operands.append(bass2jax.partition_id_tensor())
        return tuple(bass2jax._bass_exec_p.bind(
            *operands, out_avals=tuple(out_avals), in_names=tuple(all_names),
            out_names=tuple(out_names), lowering_input_output_aliases=(),
            sim_require_finite=True, sim_require_nnan=True, nc=nc))

    devices = jax.devices()[:N_CORES]
    mesh = Mesh(np.asarray(devices), ("core",))
    nio = n_params + len(out_names)
    sharded = jax.jit(
        shard_map(_body, mesh=mesh, in_specs=(PartitionSpec("core"),) * nio,
                  out_specs=(PartitionSpec("core"),) * len(out_names),
                  check_rep=False),
        donate_argnums=tuple(range(n_params, nio)), keep_unused=True)

    def run(in_maps):
        concat_in = [
            np.concatenate([np.asarray(m[nm]) for m in in_maps], axis=0)
            for nm in in_names]
        zeros = [np.zeros((N_CORES * s[0], *s[1:]), dty)
                 for s, dty in out_shapes]
        outs = sharded(*concat_in, *zeros)
        return [
            {name: np.asarray(outs[i]).reshape(N_CORES, *out_shapes[i][0])[c]
             for i, name in enumerate(out_names)}
            for c in range(N_CORES)]

    return run


def kernel(x, Wq, bq, Wk, bk, Wv, bv, Wo, bo):
    global _RUNNER
    nc, names = get_program()
    in_maps = make_in_maps(
        np.asarray(x), np.asarray(Wq), np.asarray(bq), np.asarray(Wk),
        np.asarray(bk), np.asarray(Wv), np.asarray(bv), np.asarray(Wo), names,
    )
    try:
        if _RUNNER is None:
            _RUNNER = _make_runner(nc)
        results = _RUNNER(in_maps)
    except Exception:
        from concourse.bass_utils import run_bass_kernel_spmd
        _RUNNER = None
        results = run_bass_kernel_spmd(
            nc, in_maps, core_ids=list(range(N_CORES))).results
    return combine_outputs(results, np.asarray(bo, np.float32), names)
